# revision 18
# baseline (speedup 1.0000x reference)
"""Multi-head attention block (B=4, N=2048, D=768, H=12) on 8 trn2 cores.

Sharding: core c = (batch b = c//2, head-group hg = c%2 of 6 heads).
Each core: qkv projection for its 6 heads, flash-style attention with the
score matrix kept transposed [keys, queries], partial output projection;
the host sums the two partial projections per batch and adds the bias.

Schedule (v2 -- row-tiled score matmuls):
  * Q/K are stored DUPLICATED across both partition halves
    (QT/KT [128, 6, N]: head h's 64 dh dims at partitions 0-63 AND
    64-127).  The flash score matmuls for a jc-PAIR then run as two
    CONCURRENT PE row-tiles (jc-even contracts on partitions 0-63,
    jc-odd on 64-127) -- the 64-dh contraction only needs half the PE
    rows, so pairing doubles score throughput.  The duplicate halves
    are produced by two SBUF->SBUF cross DMAs after each qk finish.
  * V projection units process all 3 pairs at once (rhs = wv [128,384])
    so each x-chunk LDWEIGHTS is amortized over 384 moving cols.
  * The flash phase is paced by ACT exp (~1.15us per [128,1024] tile).
    ACT does NOTHING but exp: the av drain is one merged DVE copy
    [65,1024] and proj finalization stages via ACT only in the
    post-flash tail (where ACT is idle).
  * qkv/proj filler micro-units pop from two paced queues (qk+V over
    the whole flash; proj it0-5 over the last ~60 slots, once both
    halves' O are normalized) to keep the PE stream dense (HAM).
  * Softmax normalization (DRAM-bounce partition spread of the ones-row
    denominator, reciprocal, broadcast, multiply) is pipelined across
    the NEXT TWO calls so the strict-FIFO DVE never waits on a DMA.
  * The LAST call (h5, half1) skips the broadcast chain: denominator
    comes back as a [128, 8] spread and 1/d is folded into the tail
    finalize via DVE scalar_tensor_tensor.

PSUM budget (8 banks of 2KB):
  sc pool  2 x [128,1024] f32 = 4 banks   (score tiles; freed by exp)
  av pool  1 x [65, 1024] f32 = 2 banks   (AV accumulator)
  fl pool  1 x [128,1024] f32 = 2 banks   (prologue/filler/tail accumulation)

Matmul convention: out = lhsT.T @ rhs, contraction on partitions; every
matmul output stays inside one 512-f32 PSUM bank (walrus rejects wider).
"""

import json
import sys

import numpy as np

sys.path.insert(0, "/opt/trn_rl_repo")

import ml_dtypes

import concourse.bass as bass
import concourse.bass2jax as bass2jax
import concourse.bass_utils as bass_utils
import concourse.tile as tile
from concourse import mybir

BF16 = ml_dtypes.bfloat16

B, N, D = 4, 2048, 768
H, DH = 12, 64
HG = 2            # head groups (cores per batch)
HL = H // HG      # heads per core
FB = HL * DH      # 384, f-dims per core
KC = D // 128     # 6 contraction chunks
JC = N // 128     # 16 key chunks
NP = HL // 2      # head pairs per core
SCALE = DH ** -0.5

# ---------------------------------------------------------------------------
# walrus in this container rejects >1 sync wait per instruction; split extra
# waits onto same-engine single-wait Drains inserted just before the owner.
# ---------------------------------------------------------------------------
_orig_compile_bir_kernel = bass_utils.compile_bir_kernel


def _split_multiwaits_json(bir_json: bytes) -> bytes:
    d = json.loads(bir_json)
    n = 0
    changed = False
    for fn in d.get("functions", []):
        for blk in fn.get("blocks", []):
            out = []
            for inst in blk["instructions"]:
                si = inst.get("sync_info") or {}
                waits = si.get("on_wait") or []
                if len(waits) > 1:
                    changed = True
                    for w in waits[:-1]:
                        n += 1
                        out.append({
                            "debug": inst.get("debug"),
                            "engine": inst["engine"],
                            "ins": [],
                            "name": f"I-wsplit-{n}",
                            "opcode": "Drain",
                            "outs": [],
                            "is_reset_sema": False,
                            "sync_info": {"on_update": [], "on_wait": [w]},
                        })
                    si["on_wait"] = [waits[-1]]
                out.append(inst)
            blk["instructions"] = out
    return json.dumps(d).encode() if changed else bir_json


def _patched_compile_bir_kernel(bir_json, tmpdir, neff_name="file.neff"):
    return _orig_compile_bir_kernel(
        _split_multiwaits_json(bir_json), tmpdir, neff_name
    )


bass_utils.compile_bir_kernel = _patched_compile_bir_kernel
bass2jax.compile_bir_kernel = _patched_compile_bir_kernel


# ---------------------------------------------------------------------------
# kernel body
# ---------------------------------------------------------------------------
def build_attention_nc(loop_iters: int | None = None) -> bass.Bass:
    f32 = mybir.dt.float32
    bf16 = mybir.dt.bfloat16
    nc = bass.Bass()
    xT = nc.declare_dram_parameter("xT", [D, N], bf16, isOutput=False)
    # per 128-chunk col layout: [Qp0 Kp0 | Qp1 Kp1 Qp2 Kp2 | V0 V1 V2]
    wT = nc.declare_dram_parameter("wT", [D, 9 * 128], bf16, isOutput=False)
    wpT = nc.declare_dram_parameter("wpT", [FB, D], bf16, isOutput=False)
    out = nc.declare_dram_parameter("out", [N, D], f32, isOutput=True)
    rl_dram = nc.dram_tensor("rl_scratch", [HL, N], f32)
    rl2_dram = nc.dram_tensor("rl2_scratch", [HL, N], f32)

    xT_r = xT.rearrange("(c p) i -> c p i", p=128)
    wT_r = wT.rearrange("(c p) m -> c p m", p=128)

    with tile.TileContext(nc) as tc:
        with (
            tc.tile_pool(name="singles", bufs=1) as singles,
            tc.tile_pool(name="epool", bufs=6) as epool,
            tc.tile_pool(name="otpool", bufs=4) as otpool,
            tc.tile_pool(name="rlbpool", bufs=3) as rlbpool,
            tc.tile_pool(name="otmpool", bufs=2) as otmpool,
            tc.tile_pool(name="outpool", bufs=4) as outpool,
            tc.tile_pool(name="scp", bufs=2, space="PSUM") as scp,
            tc.tile_pool(name="avp", bufs=1, space="PSUM") as avp,
            tc.tile_pool(name="flp", bufs=1, space="PSUM") as flp,
        ):
            def body():
                # ---- inputs, loaded per contraction chunk ----
                # wqkA = pair-0 Q|K cols (prologue-critical), wv = all V
                # cols (needed by the early all-pair V units), wqkB =
                # pair-1/2 Q|K.  x chunks split in half across the three
                # DMA-capable queues; critical stream first.
                xT_sb, wqkA_sb, wqkB_sb, wv_sb = [], [], [], []
                dmae = [nc.sync, nc.scalar, nc.gpsimd]
                for c in range(KC):
                    xc = singles.tile([128, N], bf16, name=f"xT{c}")
                    dmae[c % 3].dma_start(xc[:, 0:1024], xT_r[c][:, 0:1024])
                    xT_sb.append(xc)
                    wac = singles.tile([128, 256], bf16, name=f"wqkA{c}")
                    dmae[(c + 1) % 3].dma_start(wac[:], wT_r[c][:, 0:256])
                    wqkA_sb.append(wac)
                for c in range(KC):
                    wvc = singles.tile([128, 384], bf16, name=f"wv{c}")
                    dmae[(c + 2) % 3].dma_start(wvc[:], wT_r[c][:, 768:1152])
                    wv_sb.append(wvc)
                for c in range(KC):
                    dmae[(c + 2) % 3].dma_start(
                        xT_sb[c][:, 1024:2048], xT_r[c][:, 1024:2048])
                for c in range(KC):
                    wbc = singles.tile([128, 512], bf16, name=f"wqkB{c}")
                    dmae[c % 3].dma_start(wbc[:], wT_r[c][:, 256:768])
                    wqkB_sb.append(wbc)
                # proj weights pair-stacked: partitions = [head 2p dh | head
                # 2p+1 dh] so one proj matmul contracts BOTH heads (128).
                wp_sb = singles.tile([128, NP, D], bf16, name="wp")
                nc.sync.dma_start(
                    wp_sb[:], wpT.rearrange("(r q) e -> q r e", q=128)
                )
                # h5's tail contribution is RAW (separate psum + 1/d fold),
                # so its proj weights are also needed 64-high at rows 0-63.
                wp5_sb = singles.tile([64, D], bf16, name="wp5")
                nc.sync.dma_start(wp5_sb[:], wpT[(HL - 1) * DH:HL * DH, :])

                # ---- resident intermediates ----
                # Q/K duplicated across partition halves for row-tiling.
                QT_sb = singles.tile([128, HL, N], bf16, name="QT")
                KT_sb = singles.tile([128, HL, N], bf16, name="KT")
                V_sb = singles.tile([128, JC, HL, DH + 1], bf16, name="V")
                # O pair-stacked: even head at partitions 0-63, odd head at
                # 64-127 (shifted there by a SBUF->SBUF DMA after the
                # normalization multiply) -- one proj matmul per pair.
                O_pair = [
                    singles.tile([128, N], bf16, name=f"OP{p}")
                    for p in range(NP)
                ]
                O_raw = singles.tile([64, 1024], bf16, name="Oraw")
                rs128 = singles.tile([128, 8], f32, name="rs128")
                nc.vector.memset(V_sb[:, :, :, DH:DH + 1], 1.0)

                # ------ filler micro-units ------
                # qk unit group (t, i2): [128,1024] psum accumulated over 6
                # chunk-units (heads 2p/2p+1 stacked), then two DVE copies
                # into the matching partition halves of QT/KT plus two
                # cross SBUF->SBUF DMAs for the duplicate halves.
                # t: 0..2 = Q pairs, 3..5 = K pairs.
                def emit_qk_chunk(t, i2, c, ps):
                    pair = t % 3
                    off = 128 if t >= 3 else 0
                    if pair == 0:
                        w = wqkA_sb[c][:, off:off + 128]
                    else:
                        w = wqkB_sb[c][:, (pair - 1) * 256 + off:
                                       (pair - 1) * 256 + off + 128]
                    for s in range(2):
                        nc.tensor.matmul(
                            ps[:, s * 512:(s + 1) * 512],
                            w,
                            xT_sb[c][:, i2 * 1024 + s * 512:
                                     i2 * 1024 + (s + 1) * 512],
                            start=(c == 0),
                            stop=(c == KC - 1),
                        )

                def finish_qk(t, i2, ps):
                    dst = QT_sb if t < 3 else KT_sb
                    pair = t % 3
                    h0, h1 = 2 * pair, 2 * pair + 1
                    sl = slice(i2 * 1024, (i2 + 1) * 1024)
                    nc.vector.tensor_copy(dst[0:64, h0, sl], ps[0:64, :])
                    nc.vector.tensor_copy(dst[64:128, h1, sl], ps[64:128, :])
                    # cross dups ride the sync/pool DMA queues -- NOT the
                    # scalar queue, whose strict FIFO would head-of-line
                    # block the exp stream on the DVE-copy wait.
                    nc.gpsimd.dma_start(dst[64:128, h0, sl], dst[0:64, h0, sl])
                    nc.sync.dma_start(dst[0:64, h1, sl], dst[64:128, h1, sl])

                # V unit (jc): all 3 pairs at once -- rhs [128, 384] so the
                # 6 x-chunk LDWEIGHTS amortize over 384 moving cols.
                def emit_v_unit(jc, ps):
                    psv = ps[:, 0:384]
                    for c in range(KC):
                        nc.tensor.matmul(
                            psv,
                            xT_sb[c][:, jc * 128:(jc + 1) * 128],
                            wv_sb[c][:, 0:384],
                            start=(c == 0),
                            stop=(c == KC - 1),
                        )
                    nc.vector.tensor_copy(
                        V_sb[:, jc, :, 0:DH],
                        psv.rearrange("p (h d) -> p h d", h=HL),
                    )

                # proj unit (it, p): one head-PAIR per matmul (contraction
                # 128 over both heads' dh), 2 matmuls of 384 cols.
                def emit_proj_pair(it, p, ps, start=None, stop=None, rows=128):
                    if start is None:
                        start = (p == 0)
                    if stop is None:
                        stop = (p == NP - 1)
                    src = O_pair[p][0:rows, it * 128:(it + 1) * 128]
                    for eh in range(2):
                        nc.tensor.matmul(
                            ps[:, eh * 512:eh * 512 + 384],
                            src,
                            wp_sb[0:rows, p, eh * 384:(eh + 1) * 384],
                            start=start,
                            stop=stop,
                        )

                def emit_proj_raw5(it, ps):
                    # h5's raw contribution (last call skips normalization)
                    src = O_raw[:, (it - 8) * 128:(it - 7) * 128]
                    for eh in range(2):
                        nc.tensor.matmul(
                            ps[:, eh * 512:eh * 512 + 384],
                            src,
                            wp5_sb[:, eh * 384:(eh + 1) * 384],
                            start=True,
                            stop=True,
                        )

                def finish_proj(it, ps, use_act=False):
                    # during flash the staging copies run on DVE so ACT
                    # stays pure-exp; the post-flash tail may use ACT.
                    ob = outpool.tile([128, D], f32, tag="ob", name="ob")
                    if use_act:
                        nc.scalar.copy(ob[:, 0:384], ps[:, 0:384])
                    else:
                        nc.vector.tensor_copy(ob[:, 0:384], ps[:, 0:384])
                    nc.vector.tensor_copy(ob[:, 384:768], ps[:, 512:896])
                    nc.sync.dma_start(out[it * 128:(it + 1) * 128, :], ob[:])

                # ---- filler queue.  One FIFO (qk -> V -> proj) popped by
                # an even pacer with deadline-driven front-loading; the
                # proj units sit last so even pacing alone delays them to
                # slot ~128+, by which point both halves' O_norm exist. ----
                fillers = []
                flbox = {"ps": None}

                def flp_tile():
                    return flp.tile([128, 1024], f32, tag="fl", name="fl")

                def add_qk_units(t, i2):
                    def mk(c):
                        def u():
                            if c == 0:
                                flbox["ps"] = flp_tile()
                            emit_qk_chunk(t, i2, c, flbox["ps"])
                            if c == KC - 1:
                                finish_qk(t, i2, flbox["ps"])
                                flbox["ps"] = None
                        return u
                    for c in range(KC):
                        fillers.append(mk(c))

                def add_v_unit(jc):
                    def u():
                        ps = flp_tile()
                        emit_v_unit(jc, ps)
                    fillers.append(u)

                def add_proj_units(it):
                    def mk(p):
                        def u():
                            if p == 0:
                                flbox["ps"] = flp_tile()
                            emit_proj_pair(it, p, flbox["ps"])
                            if p == NP - 1:
                                finish_proj(it, flbox["ps"])
                                flbox["ps"] = None
                        return u
                    for p in range(NP):
                        fillers.append(mk(p))

                popped = [0]

                def popper(slot):
                    # Unit indices (91 total):
                    #  0..5    K(3,1)      by slot ~2 (scores jc>=8 + dup DMA)
                    #  6..18   V jc3..15   (jc k by slot k-1)
                    #  19..30  Q(1,0) K(4,0)  by slot 29 (call 2 = h2)
                    #  31..36  K(4,1)         by slot 37
                    #  37..48  Q(2,0) K(5,0)  by slot 61 (call 4 = h4)
                    #  49..54  K(5,1)         by slot 69
                    #  55..60  Q(0,1)         by slot 93 (call 6)
                    #  61..66  Q(1,1)         by slot 125 (call 8)
                    #  67..72  Q(2,1)         by slot 157 (call 10)
                    #  73..90  proj it0..5    (even pacing alone reaches
                    #          these from slot ~155, after O_pair exists)
                    need = min(6, 2 * slot + 2)
                    if slot >= 1:
                        need = max(need, min(19, slot + 5))
                    for dl, tot in ((29, 31), (37, 37), (61, 49), (69, 55),
                                    (93, 61), (125, 67), (157, 73)):
                        if slot >= dl:
                            need = max(need, tot)
                    tgt = max(need, (91 * (slot + 1) + 191) // 192)
                    while popped[0] < tgt and fillers:
                        fillers.pop(0)()
                        popped[0] += 1

                # ---- prologue: the true critical set for flash (h0,
                # half0): K-pair0 first half, Q-pair0 first half (both with
                # their dup DMAs), V jc0..5.  Chunk-OUTER so compute starts
                # as soon as input chunk 0 lands. ----
                pro_keys = ((3, 0), (0, 0))
                pro = {
                    pro_keys[0]: scp.tile([128, 1024], f32, tag="sc",
                                          name="ps_pro"),
                    pro_keys[1]: flp_tile(),
                }
                for c in range(KC):
                    for t, i2 in pro_keys:
                        emit_qk_chunk(t, i2, c, pro[(t, i2)])
                for t, i2 in pro_keys:
                    finish_qk(t, i2, pro[(t, i2)])
                for jc in range(3):
                    ps = scp.tile([128, 1024], f32, tag="sc", name="ps_pro")
                    emit_v_unit(jc, ps)

                # filler queue composition (indices documented in popper)
                add_qk_units(3, 1)
                for jc in range(3, JC):
                    add_v_unit(jc)
                for t in (1, 4):
                    add_qk_units(t, 0)
                add_qk_units(4, 1)
                for t in (2, 5):
                    add_qk_units(t, 0)
                add_qk_units(5, 1)
                add_qk_units(0, 1)
                add_qk_units(1, 1)
                add_qk_units(2, 1)
                for it in range(6):
                    add_proj_units(it)

                # ---- normalization pipeline: the DRAM-bounce chain is
                # split into stages run 1-2 calls later so that no strict-
                # FIFO engine (DVE) ever waits on an in-flight DMA. ----
                norm_a, norm_b = [], []   # deferred stage closures

                def norm_stage_a(h, q0, rs, ot):
                    nc.vector.reciprocal(rs[:], rs[:])
                    nc.sync.dma_start(
                        rl2_dram[h, q0:q0 + 1024].rearrange("(p c) -> p c", p=64),
                        rs[:],
                    )
                    rlb = rlbpool.tile([64, 1024], f32, tag="rlb")
                    rl_src = rl2_dram[h, q0:q0 + 1024]
                    rl_bcast = bass.AP(
                        tensor=rl_src.tensor,
                        offset=rl_src.offset,
                        ap=[[0, 64]] + list(rl_src.ap),
                    )
                    nc.sync.dma_start(rlb[:], rl_bcast)
                    norm_b.append(lambda: norm_stage_b(h, q0, rlb, ot))

                def norm_stage_b(h, q0, rlb, ot):
                    # final multiply on the otherwise-idle Pool engine.
                    # Odd heads land in a temp tile (engines are lane-
                    # aligned) and a SBUF->SBUF DMA shifts them to
                    # partitions 64-127 of the pair tile.
                    p = h // 2
                    if h % 2 == 0:
                        nc.gpsimd.tensor_mul(
                            O_pair[p][0:64, q0:q0 + 1024], ot[0:DH, :], rlb[:]
                        )
                    else:
                        om = otmpool.tile([64, 1024], bf16, tag="om")
                        nc.gpsimd.tensor_mul(om[:], ot[0:DH, :], rlb[:])
                        nc.gpsimd.dma_start(
                            O_pair[p][64:128, q0:q0 + 1024], om[:]
                        )

                def run_pending(stage):
                    while stage:
                        stage.pop(0)()

                # ---- flash call: one head, one 1024-query half.  Score
                # matmuls are emitted as row-tiled jc-PAIRS: jc-even
                # contracts on partitions 0-63, jc-odd on 64-127, running
                # concurrently on disjoint PE subarrays. ----
                def flash(h, half, slot0, last=False):
                    q0 = half * 1024
                    av = avp.tile([DH + 1, 1024], f32, tag="av", name="av")
                    for jp in range(JC // 2):
                        jc0, jc1 = 2 * jp, 2 * jp + 1
                        ps0 = scp.tile([128, 1024], f32, tag="sc",
                                       name="ps_sc")
                        ps1 = scp.tile([128, 1024], f32, tag="sc",
                                       name="ps_sc")
                        for q in range(2):
                            nc.tensor.matmul(
                                ps0[:, q * 512:(q + 1) * 512],
                                KT_sb[0:64, h, jc0 * 128:(jc0 + 1) * 128],
                                QT_sb[0:64, h,
                                      q0 + q * 512:q0 + (q + 1) * 512],
                                start=True,
                                stop=True,
                            )
                            nc.tensor.matmul(
                                ps1[:, q * 512:(q + 1) * 512],
                                KT_sb[64:128, h, jc1 * 128:(jc1 + 1) * 128],
                                QT_sb[64:128, h,
                                      q0 + q * 512:q0 + (q + 1) * 512],
                                start=True,
                                stop=True,
                            )
                        for jc, ps in ((jc0, ps0), (jc1, ps1)):
                            et = epool.tile([128, 1024], bf16, tag="e")
                            nc.scalar.activation(
                                et[:], ps[:],
                                mybir.ActivationFunctionType.Exp,
                                scale=float(SCALE),
                            )
                            if jc == 2:
                                run_pending(norm_b)
                            elif jc == 6:
                                run_pending(norm_a)
                            popper(slot0 + jc)
                            for q in range(2):
                                nc.tensor.matmul(
                                    av[:, q * 512:(q + 1) * 512],
                                    V_sb[:, jc, h, :],
                                    et[:, q * 512:(q + 1) * 512],
                                    start=(jc == 0),
                                    stop=(jc == JC - 1),
                                )
                    ot = otpool.tile([DH + 1, 1024], f32, tag="ot")
                    if last:
                        # fast path for the final call: keep O' raw (bf16)
                        # and bounce the denominator back as a [128, 8]
                        # per-partition spread -- the tail folds 1/d into
                        # the proj finalize, so no broadcast chain exists.
                        nc.vector.tensor_copy(ot[DH:DH + 1, :], av[DH:DH + 1, :])
                        nc.vector.tensor_copy(O_raw[:], av[0:DH, :])
                        nc.sync.dma_start(
                            rl_dram[h:h + 1, q0:q0 + 1024], ot[DH:DH + 1, :]
                        )
                        nc.sync.dma_start(
                            rs128[:],
                            rl_dram[h, q0:q0 + 1024].rearrange(
                                "(c p) -> p c", p=128),
                        )
                        nc.vector.reciprocal(rs128[:], rs128[:])
                        return
                    # merged drain: one DVE copy releases av, then the
                    # denominator row bounces through DRAM as a [64,16]
                    # partition spread; recip runs next call.
                    nc.vector.tensor_copy(ot[:], av[:])
                    nc.sync.dma_start(
                        rl_dram[h:h + 1, q0:q0 + 1024], ot[DH:DH + 1, :]
                    )
                    rs = rlbpool.tile([64, 16], f32, tag="rs")
                    nc.sync.dma_start(
                        rs[:],
                        rl_dram[h, q0:q0 + 1024].rearrange("(p c) -> p c", p=64),
                    )
                    norm_a.append(lambda: norm_stage_a(h, q0, rs, ot))

                for h in range(HL):
                    flash(h, 0, h * JC)
                for h in range(HL):
                    flash(h, 1, HL * JC + h * JC, last=(h == HL - 1))
                while fillers:
                    fillers.pop(0)()
                run_pending(norm_a)
                run_pending(norm_b)
                # ---- tail: proj its 6..15.  its 6,7 (half-0 tokens) use
                # the plain finalize; its 8..15: h0..h4 accumulate via the
                # normalized O_norm, h5 contributes RAW via a separate psum
                # tile, scaled by the per-partition reciprocal in the DVE
                # finalize. ----
                for it in range(6, 8):
                    ps = scp.tile([128, 1024], f32, tag="sc", name="ps_tail")
                    for p in range(NP):
                        emit_proj_pair(it, p, ps)
                    finish_proj(it, ps, use_act=True)
                for it in range(8, 16):
                    ps = scp.tile([128, 1024], f32, tag="sc", name="ps_tail")
                    ps5 = (flp_tile() if it % 2 == 0 else
                           avp.tile([128, 1024], f32, tag="av", name="ps_t5"))
                    emit_proj_pair(it, 0, ps, start=True, stop=False)
                    emit_proj_pair(it, 1, ps, start=False, stop=False)
                    # pair 2: only h4 (rows 0-63) -- h5 is raw, below
                    emit_proj_pair(it, 2, ps, start=False, stop=True, rows=64)
                    emit_proj_raw5(it, ps5)
                    ob = outpool.tile([128, D], f32, tag="ob", name="ob")
                    rcol = rs128[:, it - 8:it - 7]
                    # DVE reads at most one PSUM operand: ACT stages ps into
                    # ob, DVE folds ps5 * (1/d5) on top.
                    nc.scalar.copy(ob[:, 0:384], ps[:, 0:384])
                    nc.vector.tensor_copy(ob[:, 384:768], ps[:, 512:896])
                    nc.vector.scalar_tensor_tensor(
                        ob[:, 0:384], ps5[:, 0:384], rcol, ob[:, 0:384],
                        mybir.AluOpType.mult, mybir.AluOpType.add,
                    )
                    nc.vector.scalar_tensor_tensor(
                        ob[:, 384:768], ps5[:, 512:896], rcol, ob[:, 384:768],
                        mybir.AluOpType.mult, mybir.AluOpType.add,
                    )
                    nc.sync.dma_start(out[it * 128:(it + 1) * 128, :], ob[:])

            if loop_iters is None:
                body()
            else:
                with tc.For_i(0, loop_iters, 1):
                    body()

    return nc


# ---------------------------------------------------------------------------
# host-side runner (cached jitted executable, per-core input packing)
# ---------------------------------------------------------------------------
_RUNNER = None


def _get_runner():
    global _RUNNER
    if _RUNNER is None:
        nc = build_attention_nc()
        _RUNNER = _make_runner(nc, n_cores=8)
    return _RUNNER


def _make_runner(nc, n_cores):
    """Build the sharded jitted executable once (mirrors run_bass_via_pjrt)."""
    import jax
    from jax.sharding import Mesh, PartitionSpec
    from jax.experimental.shard_map import shard_map

    bass2jax.install_neuronx_cc_hook()

    partition_name = (
        nc.partition_id_tensor.name if nc.partition_id_tensor else None
    )
    in_names, out_names, out_avals, zero_outs = [], [], [], []
    for alloc in nc.m.functions[0].allocations:
        if not isinstance(alloc, mybir.MemoryLocationSet):
            continue
        name = alloc.memorylocations[0].name
        if alloc.kind == "ExternalInput":
            if name != partition_name:
                in_names.append(name)
        elif alloc.kind == "ExternalOutput":
            out_names.append(name)
            shape = tuple(alloc.tensor_shape)
            dtype = mybir.dt.np(alloc.dtype)
            out_avals.append(jax.core.ShapedArray(shape, dtype))
            zero_outs.append(np.zeros(shape, dtype))
    n_params = len(in_names)
    all_in_names = in_names + out_names
    if partition_name is not None:
        all_in_names = all_in_names + [partition_name]

    def _body(*args):
        operands = list(args)
        if partition_name is not None:
            operands.append(bass2jax.partition_id_tensor())
        outs = bass2jax._bass_exec_p.bind(
            *operands,
            out_avals=tuple(out_avals),
            in_names=tuple(all_in_names),
            out_names=tuple(out_names),
            lowering_input_output_aliases=(),
            sim_require_finite=True,
            sim_require_nnan=True,
            nc=nc,
        )
        return tuple(outs)

    devices = jax.devices()[:n_cores]
    mesh = Mesh(np.asarray(devices), ("core",))
    n_outs = len(out_names)
    sharded = jax.jit(
        shard_map(
            _body,
            mesh=mesh,
            in_specs=(PartitionSpec("core"),) * (n_params + n_outs),
            out_specs=(PartitionSpec("core"),) * n_outs,
            check_rep=False,
        ),
        donate_argnums=tuple(range(n_params, n_params + n_outs)),
        keep_unused=True,
    )

    def pack(in_maps):
        concat_in = [
            np.concatenate([np.asarray(m[name]) for m in in_maps], axis=0)
            for name in in_names
        ]
        concat_zero = [
            np.zeros((n_cores * z.shape[0], *z.shape[1:]), z.dtype)
            for z in zero_outs
        ]
        return concat_in, concat_zero

    def unpack(out_arrs):
        return [
            {
                name: np.asarray(out_arrs[i]).reshape(
                    n_cores, *out_avals[i].shape
                )[c]
                for i, name in enumerate(out_names)
            }
            for c in range(n_cores)
        ]

    def run(in_maps):
        concat_in, concat_zero = pack(in_maps)
        return unpack(sharded(*concat_in, *concat_zero))

    run.in_names = in_names
    run.out_names = out_names
    run.pack = pack
    run.unpack = unpack
    run.sharded = sharded
    run.mesh = mesh
    return run


def make_in_maps(x, w_qkv, w_proj):
    """Shard/pack full inputs into the 8 per-core input maps."""
    W = np.ascontiguousarray(w_qkv).reshape(3, H, DH, D)
    in_maps = []
    for c in range(8):
        b, hg = c // HG, c % HG
        blocks = []
        # [Qp0 Kp0 | Qp1 Kp1 Qp2 Kp2 | V0 V1 V2]
        for p in range(NP):
            h0 = hg * HL + 2 * p
            blocks.append(W[0, h0:h0 + 2].reshape(2 * DH, D))
            blocks.append(W[1, h0:h0 + 2].reshape(2 * DH, D))
        for p in range(NP):
            h0 = hg * HL + 2 * p
            blocks.append(W[2, h0:h0 + 2].reshape(2 * DH, D))
        wg = np.concatenate(blocks, axis=0)                        # [1152, 768]
        in_maps.append({
            "xT": np.ascontiguousarray(x[b].T).astype(BF16),
            "wT": np.ascontiguousarray(wg.T).astype(BF16),
            "wpT": np.ascontiguousarray(
                w_proj[:, hg * FB:(hg + 1) * FB].T).astype(BF16),
        })
    return in_maps


def kernel(x, w_qkv, w_proj, b_proj):
    x = np.asarray(x, dtype=np.float32)
    w_qkv = np.asarray(w_qkv, dtype=np.float32)
    w_proj = np.asarray(w_proj, dtype=np.float32)
    b_proj = np.asarray(b_proj, dtype=np.float32)

    run = _get_runner()
    results = run(make_in_maps(x, w_qkv, w_proj))

    out = np.empty((B, N, D), dtype=np.float32)
    for b in range(B):
        out[b] = results[2 * b]["out"] + results[2 * b + 1]["out"] + b_proj
    return out
